# revision 4
# baseline (speedup 1.0000x reference)
"""Multi-head attention (B=2, S=2048, E=1024, H=16, D=64) on 8 Trainium2 cores.

Sharding: data-parallel over batch (2 groups of 4 cores), tensor-parallel over
heads within each group (4 heads per core, Megatron-style column-split qkv).
out_proj is sharded over its INPUT rows: each core multiplies its own 256
attention-output rows (4 heads x 64) by w_out[r*256:(r+1)*256, :] and emits a
full-width partial product y_r [2048, 1024]; the host sums the 4 partials per
batch during unsharding (no device collective). The SPMD program has no
core-id logic.

The kernel is paced by ScalarE: softmax exp over 4 x 2048 x 2048 scores per
core is ~157 us of ACT time (1 elem/lane/cycle @ 1.2 GHz), while the PE work
(projections + scores + PV + out_proj) is ~140-180 us. v1 serialized ~50 us
of qkv projections before the first exp because projections and score tiles
shared one 2-buf PSUM ring. v2 keeps ScalarE continuously fed:
  - Only kT(span0) + qT(span0) + v(span0) are emitted before the attention
    loop; the remaining projection chains are woven one-per-2-kc-iterations
    into qc0..qc2's score/PV stream (same shared "mm" ring, now in
    consumption order).
  - expp (exp output pool) is 14-deep so ACT streams through qc boundaries
    while PV waits for the fz/ep PSUM ring to recycle behind normalize +
    out_proj.
  - normalize is split per-128-row chunk so out_proj's first chunk starts
    ~2.5 us after the last PV instead of ~6 us.
  - for reps>1 (timing NEFFs), the next rep's xT DMA + span0 projections are
    woven into qc3 so the exp stream crosses rep boundaries with ~no gap.

All PE inputs bf16, fp32 PSUM accumulation; x arrives host-pre-transposed
(x^T [1024, 2048] bf16). Scores per (qc, kc, head-pair) are two K=64 matmuls
row-packed at base partitions 0/64 (PE row-tiles 2x when HW concurrency
applies). PV uses lhsT=[v_h | ones] so softmax denominators accumulate in
PSUM partitions 64..127 for free. PSUM: "mm" 2x[128,1024] (proj chains,
score tiles) + "fz" 2x[128,2,512] (PV accumulators, out_proj partials)
= exactly 8 banks.
"""

import numpy as np
from contextlib import ExitStack

import concourse.tile as tile
from concourse import bacc, mybir
from concourse.bass_utils import run_bass_kernel_spmd

B, S, E, H, D = 2, 2048, 1024, 16, 64
N_CORES = 8
HPC = 4            # heads per core
HD = HPC * D       # 256

F32 = mybir.dt.float32
BF16 = mybir.dt.bfloat16
EXP = mybir.ActivationFunctionType.Exp

_cached = None


class _T:
    pass


def build(reps=1, profile=False):
    nc = bacc.Bacc("TRN2", target_bir_lowering=False, debug=False,
                   num_devices=N_CORES)

    T = _T()
    T.xT_d = nc.dram_tensor("xT", [E, S], BF16, kind="ExternalInput").ap()
    wq_d = nc.dram_tensor("wq", [E, HD], BF16, kind="ExternalInput").ap()
    wk_d = nc.dram_tensor("wk", [E, HD], BF16, kind="ExternalInput").ap()
    wv_d = nc.dram_tensor("wv", [E, HD], BF16, kind="ExternalInput").ap()
    wo_d = nc.dram_tensor("wo", [HD, E], BF16, kind="ExternalInput").ap()
    T.y_d = nc.dram_tensor("y", [S, E], F32, kind="ExternalOutput").ap()

    with tile.TileContext(nc) as tc, ExitStack() as ctx:
        glob = ctx.enter_context(tc.tile_pool(name="glob", bufs=1))
        T.xT = glob.tile([128, 8, S], BF16, tag="xT")
        T.wq = glob.tile([128, 8, HD], BF16, tag="wq")
        T.wk = glob.tile([128, 8, HD], BF16, tag="wk")
        T.wv = glob.tile([128, 8, HD], BF16, tag="wv")
        T.wo = glob.tile([128, 2, E], BF16, tag="wo")
        T.qT = glob.tile([128, 2, S], BF16, tag="qT")   # q^T by head pair
        T.kT = glob.tile([128, 2, S], BF16, tag="kT")
        T.v = glob.tile([128, 16, HPC, 128], BF16, tag="v")  # [v_h | ones]
        T.xT_r = T.xT_d.rearrange("(c p) s -> p c s", p=128)

        nc.sync.dma_start(T.wk[:], wk_d.rearrange("(c p) n -> p c n", p=128))
        nc.sync.dma_start(T.wq[:], wq_d.rearrange("(c p) n -> p c n", p=128))
        nc.sync.dma_start(T.wv[:], wv_d.rearrange("(c p) n -> p c n", p=128))
        nc.sync.dma_start(T.wo[:], wo_d.rearrange("(c p) n -> p c n", p=128))
        # ones half of every v_aug block; v halves overwritten per rep
        nc.gpsimd.memset(T.v[:], 1.0)

        # tiny dep-free exp so the ~2.7us ACT table load runs at t~0
        warm_in = glob.tile([1, 8], F32, tag="warm_in")
        warm_out = glob.tile([1, 8], BF16, tag="warm_out")
        nc.gpsimd.memset(warm_in[:], 0.0)
        nc.scalar.activation(warm_out[:], warm_in[:], EXP, scale=1.0)

        with ExitStack() as body:
            T.mm = body.enter_context(
                tc.tile_pool(name="mm", bufs=2, space="PSUM"))
            T.fzp = body.enter_context(
                tc.tile_pool(name="fzp", bufs=2, space="PSUM"))
            T.expp = body.enter_context(tc.tile_pool(name="expp", bufs=14))
            T.recp = body.enter_context(tc.tile_pool(name="recp", bufs=4))
            T.outp = body.enter_context(tc.tile_pool(name="outp", bufs=2))
            T.ysb = body.enter_context(tc.tile_pool(name="ysb", bufs=3))

            for rep in range(reps):
                _emit_body(nc, T, first=(rep == 0), last=(rep == reps - 1))

    nc.compile()
    return nc


def _emit_body(nc, T, first, last):
    def dma_spans():
        for z in range(4):
            nc.sync.dma_start(T.xT[:, :, z * 512:(z + 1) * 512],
                              T.xT_r[:, :, z * 512:(z + 1) * 512])

    def u_proj_T(w_t, dst, z, mc):
        # dst[:, mc, span z] = (w col-block mc)^T @ x^T span z
        def emit():
            pp = T.mm.tile([128, 1024], F32, tag="mm")
            for ec in range(8):
                nc.tensor.matmul(pp[:, 0:512],
                                 w_t[:, ec, mc * 128:(mc + 1) * 128],
                                 T.xT[:, ec, z * 512:(z + 1) * 512],
                                 start=(ec == 0), stop=(ec == 7))
            nc.vector.tensor_copy(dst[:, mc, z * 512:(z + 1) * 512],
                                  pp[:, 0:512])
        return emit

    def u_v(sc0):
        # v rows for seq chunks sc0, sc0+1 (two 8-matmul chains, one mm tile)
        def emit():
            pp = T.mm.tile([128, 1024], F32, tag="mm")
            for j in range(2):
                sc = sc0 + j
                for ec in range(8):
                    nc.tensor.matmul(pp[:, j * HD:(j + 1) * HD],
                                     T.xT[:, ec, sc * 128:(sc + 1) * 128],
                                     T.wv[:, ec, :],
                                     start=(ec == 0), stop=(ec == 7))
            for j in range(2):
                sc = sc0 + j
                nc.vector.tensor_copy(
                    T.v[:, sc, :, 0:64],
                    pp[:, j * HD:(j + 1) * HD].rearrange(
                        "p (h d) -> p h d", h=HPC))
        return emit

    if first:
        dma_spans()
        for emit in (u_proj_T(T.wk, T.kT, 0, 0), u_proj_T(T.wk, T.kT, 0, 1),
                     u_proj_T(T.wq, T.qT, 0, 0), u_proj_T(T.wq, T.qT, 0, 1),
                     u_v(0), u_v(2)):
            emit()

    # per-qc weave lists (consumption-ordered; deadline = kc 4z for span z)
    weave = {
        0: [u_proj_T(T.wk, T.kT, 1, 0), u_proj_T(T.wk, T.kT, 1, 1),
            u_v(4), u_v(6),
            u_proj_T(T.wk, T.kT, 2, 0), u_proj_T(T.wk, T.kT, 2, 1),
            u_v(8), u_v(10),
            u_proj_T(T.wk, T.kT, 3, 0), u_proj_T(T.wk, T.kT, 3, 1),
            u_v(12), u_v(14),
            u_proj_T(T.wq, T.qT, 1, 0), u_proj_T(T.wq, T.qT, 1, 1)],
        1: [u_proj_T(T.wq, T.qT, 2, 0), u_proj_T(T.wq, T.qT, 2, 1)],
        2: [u_proj_T(T.wq, T.qT, 3, 0), u_proj_T(T.wq, T.qT, 3, 1)],
        3: [] if last else [
            u_proj_T(T.wk, T.kT, 0, 0), u_proj_T(T.wk, T.kT, 0, 1),
            u_proj_T(T.wq, T.qT, 0, 0), u_proj_T(T.wq, T.qT, 0, 1),
            u_v(0), u_v(2)],
    }

    for qc in range(4):
        units = weave[qc]
        ui = 0
        fzs = None
        for kc in range(16):
            if qc == 3 and kc == 0 and not last:
                dma_spans()   # next rep's x^T; WAR-safe (readers done by qc2)
            sts = []
            for hp in range(2):
                st = T.mm.tile([128, 1024], F32, tag="mm")
                for par in range(2):   # K=64 row-packed head pair
                    lo, hi = par * 64, (par + 1) * 64
                    nc.tensor.matmul(
                        st[:, par * 512:(par + 1) * 512],
                        T.kT[lo:hi, hp, kc * 128:(kc + 1) * 128],
                        T.qT[lo:hi, hp, qc * 512:(qc + 1) * 512],
                        start=True, stop=True)
                sts.append(st)
            exs = []
            for hp in range(2):
                ex = T.expp.tile([128, 1024], BF16, tag="ex")
                nc.scalar.activation(ex[:], sts[hp][:], EXP, scale=0.125)
                exs.append(ex)
            if kc == 0:
                fz0 = T.fzp.tile([128, 2, 512], F32, tag="fz")
                fz1 = T.fzp.tile([128, 2, 512], F32, tag="fz")
                fzs = [fz0, fz1]
            for hp in range(2):
                for par in range(2):
                    h = 2 * hp + par
                    nc.tensor.matmul(
                        fzs[hp][:, par, :],
                        T.v[:, kc, h, :],
                        exs[hp][:, par * 512:(par + 1) * 512],
                        start=(kc == 0), stop=(kc == 15))
            if kc >= 1 and ui < len(units):
                units[ui]()
                ui += 1
        while ui < len(units):
            units[ui]()
            ui += 1

        # normalize: big recips, then per-sq muls so ep(sq0) unblocks early
        outT = T.outp.tile([128, 2, 512], BF16, tag="outT")
        rcs = {}
        for hp in range(2):
            for par in range(2):
                rc = T.recp.tile([64, 512], F32, tag="rc")
                nc.vector.reciprocal(rc[:], fzs[hp][64:128, par, :])
                rcs[(hp, par)] = rc
        for sq in range(4):
            for hp in range(2):
                for par in range(2):
                    nc.vector.tensor_mul(
                        outT[par * 64:(par + 1) * 64, hp,
                             sq * 128:(sq + 1) * 128],
                        fzs[hp][0:64, par, sq * 128:(sq + 1) * 128],
                        rcs[(hp, par)][:, sq * 128:(sq + 1) * 128])

        # out_proj partials per 128-row chunk; fz ring slots recycle behind
        for sq in range(4):
            ep = T.fzp.tile([128, 2, 512], F32, tag="fz")
            for half in range(2):     # matmul dst must fit one PSUM bank
                for hp in range(2):
                    nc.tensor.matmul(
                        ep[:, half, :],
                        outT[:, hp, sq * 128:(sq + 1) * 128],
                        T.wo[:, hp, half * 512:(half + 1) * 512],
                        start=(hp == 0), stop=(hp == 1))
            yt = T.ysb.tile([128, E], F32, tag="y")
            nc.vector.tensor_copy(yt[:], ep.rearrange("p a b -> p (a b)"))
            nc.sync.dma_start(
                T.y_d[(qc * 4 + sq) * 128:(qc * 4 + sq + 1) * 128, :], yt[:])


def _get_nc():
    global _cached
    if _cached is None:
        _cached = build()
    return _cached


def _bf16(a):
    import ml_dtypes
    return np.ascontiguousarray(a, dtype=ml_dtypes.bfloat16)


def make_in_maps(x, w_qkv, w_out):
    x = np.asarray(x, dtype=np.float32)
    w_qkv = np.asarray(w_qkv, dtype=np.float32)
    w_out = np.asarray(w_out, dtype=np.float32)
    in_maps = []
    for c in range(N_CORES):
        b, r = c // 4, c % 4
        hs = r * HD                  # first qkv column of this core's heads
        in_maps.append({
            "xT": _bf16(x[b].T),
            "wq": _bf16(w_qkv[:, hs:hs + HD]),
            "wk": _bf16(w_qkv[:, E + hs:E + hs + HD]),
            "wv": _bf16(w_qkv[:, 2 * E + hs:2 * E + hs + HD]),
            "wo": _bf16(w_out[r * HD:(r + 1) * HD, :]),
        })
    return in_maps


def assemble(results):
    y = np.zeros((B, S, E), dtype=np.float32)
    for c in range(N_CORES):
        y[c // 4] += results[c]["y"]
    return y


def kernel(x, w_qkv, w_out):
    nc = _get_nc()
    res = run_bass_kernel_spmd(nc, make_in_maps(x, w_qkv, w_out),
                               list(range(N_CORES)))
    return assemble(res.results)


# revision 9
# speedup vs baseline: 1.1328x; 1.1328x over previous
"""Multi-head attention (B=2, S=2048, E=1024, H=16, D=64) on 8 Trainium2 cores.

Sharding: data-parallel over batch (2 groups of 4 cores), tensor-parallel
over heads within each group (4 heads per core, Megatron-style). out_proj is
sharded over its input rows; each core emits a full-width partial y and the
host sums 4 partials per batch (no device collective, no core-id logic).

The kernel is paced by the ScalarE exp stream: 128 x exp([128,1024] PSUM ->
bf16 SBUF) at ~0.7us each. v5's key fix over v1 is PE-queue ordering: the PE
is in-order, and v1 emitted PV(i) (which waits on exp(i)) before
scores(i+1), so every kc-iteration serialized PE behind ACT
(~1.3us/exp measured). v5 defers PV by 2 iterations - the PE stream is
[scores(i), PV(i-2)] - so no PE instruction ever waits on a
recently-issued exp, and the loop runs at the engines' throughput instead
of the dependency-chain latency (measured 165 -> ~95us for the bare loop).

Also:
  - qkv projections are woven into the attention loop (one 8-matmul chain
    between iterations) instead of a serial ~40us head; for reps>1 the next
    rep's x^T DMA + span-0 projections weave into qc3.
  - exp-output pool is 18 deep so the ACT stream rides through the
    normalize/out_proj PSUM-ring recycling at q-chunk boundaries.
  - normalize uses 2 reciprocals [64,2,512] + 4 muls (DVE instr count kept
    low; DVE ops cost ~0.6-1.2us each).
  - scores: two K=64 heads row-packed at base partitions 0/64 run
    concurrently on the PE (row tiling), par0 always writing bank0 and par1
    bank1 of the score tile (concurrent row-tiled streams must never cross
    PSUM banks - hardware hazard).
  - PV lhsT = [v_h | ones]: softmax denominators accumulate free in
    partitions 64..127.
PSUM: "st" 2x[128,1024] (scores) + "fz" 2x[128,2,512] (PV accumulators +
out_proj partials) = 8 banks. All PE inputs bf16, f32 accumulation; x
arrives host-pre-transposed; exp(scale=1/8), no max-subtraction.
"""

import numpy as np
from contextlib import ExitStack

import concourse.tile as tile
from concourse import bacc, mybir
from concourse.bass_utils import run_bass_kernel_spmd

B, S, E, H, D = 2, 2048, 1024, 16, 64
N_CORES = 8
HPC = 4            # heads per core
HD = HPC * D       # 256
DELTA = 2          # PV deferral depth (iterations)

F32 = mybir.dt.float32
BF16 = mybir.dt.bfloat16
EXP = mybir.ActivationFunctionType.Exp

_cached = None


class _T:
    pass


def build(reps=1, profile=False):
    nc = bacc.Bacc("TRN2", target_bir_lowering=False, debug=False,
                   num_devices=N_CORES)

    T = _T()
    T.xT_d = nc.dram_tensor("xT", [E, S], BF16, kind="ExternalInput").ap()
    wq_d = nc.dram_tensor("wq", [E, HD], BF16, kind="ExternalInput").ap()
    wk_d = nc.dram_tensor("wk", [E, HD], BF16, kind="ExternalInput").ap()
    wv_d = nc.dram_tensor("wv", [E, HD], BF16, kind="ExternalInput").ap()
    wo_d = nc.dram_tensor("wo", [HD, E], BF16, kind="ExternalInput").ap()
    T.y_d = nc.dram_tensor("y", [S, E], F32, kind="ExternalOutput").ap()

    with tile.TileContext(nc) as tc, ExitStack() as ctx:
        glob = ctx.enter_context(tc.tile_pool(name="glob", bufs=1))
        T.xT = glob.tile([128, 8, S], BF16, tag="xT")
        T.wq = glob.tile([128, 8, HD], BF16, tag="wq")
        T.wk = glob.tile([128, 8, HD], BF16, tag="wk")
        T.wv = glob.tile([128, 8, HD], BF16, tag="wv")
        T.wo = glob.tile([128, 2, E], BF16, tag="wo")
        T.qT = glob.tile([128, 2, S], BF16, tag="qT")   # q^T by head pair
        T.kT = glob.tile([128, 2, S], BF16, tag="kT")
        T.v = glob.tile([128, 16, HPC, 128], BF16, tag="v")  # [v_h | ones]
        T.xT_r = T.xT_d.rearrange("(c p) s -> p c s", p=128)

        nc.sync.dma_start(T.wk[:], wk_d.rearrange("(c p) n -> p c n", p=128))
        nc.sync.dma_start(T.wq[:], wq_d.rearrange("(c p) n -> p c n", p=128))
        nc.sync.dma_start(T.wv[:], wv_d.rearrange("(c p) n -> p c n", p=128))
        nc.sync.dma_start(T.wo[:], wo_d.rearrange("(c p) n -> p c n", p=128))
        nc.gpsimd.memset(T.v[:], 1.0)   # ones halves; v halves written/rep

        # tiny dep-free exp so the ~2.7us ACT table load runs at t~0
        warm_in = glob.tile([1, 8], F32, tag="warm_in")
        warm_out = glob.tile([1, 8], BF16, tag="warm_out")
        nc.gpsimd.memset(warm_in[:], 0.0)
        nc.scalar.activation(warm_out[:], warm_in[:], EXP, scale=1.0)

        with ExitStack() as body:
            T.mm = body.enter_context(
                tc.tile_pool(name="mm", bufs=2, space="PSUM"))
            T.fzp = body.enter_context(
                tc.tile_pool(name="fzp", bufs=2, space="PSUM"))
            T.expp = body.enter_context(tc.tile_pool(name="expp", bufs=18))
            T.recp = body.enter_context(tc.tile_pool(name="recp", bufs=4))
            T.outp = body.enter_context(tc.tile_pool(name="outp", bufs=2))
            T.ysb = body.enter_context(tc.tile_pool(name="ysb", bufs=3))

            for rep in range(reps):
                _emit_body(nc, T, first=(rep == 0), last=(rep == reps - 1))

    nc.compile()
    return nc


def _emit_body(nc, T, first, last):
    def dma_spans():
        for z in range(4):
            nc.sync.dma_start(T.xT[:, :, z * 512:(z + 1) * 512],
                              T.xT_r[:, :, z * 512:(z + 1) * 512])

    def u_proj_T(w_t, dst, z, mc):
        # dst[:, mc, span z] = (w col-block mc)^T @ x^T span z
        def emit():
            pp = T.mm.tile([128, 1024], F32, tag="st")
            for ec in range(8):
                nc.tensor.matmul(pp[:, 0:512],
                                 w_t[:, ec, mc * 128:(mc + 1) * 128],
                                 T.xT[:, ec, z * 512:(z + 1) * 512],
                                 start=(ec == 0), stop=(ec == 7))
            nc.vector.tensor_copy(dst[:, mc, z * 512:(z + 1) * 512],
                                  pp[:, 0:512])
        return emit

    def u_v(sc0):
        # v rows for seq chunks sc0, sc0+1; one merged copy
        def emit():
            pp = T.mm.tile([128, 1024], F32, tag="st")
            for j in range(2):
                sc = sc0 + j
                for ec in range(8):
                    nc.tensor.matmul(pp[:, j * HD:(j + 1) * HD],
                                     T.xT[:, ec, sc * 128:(sc + 1) * 128],
                                     T.wv[:, ec, :],
                                     start=(ec == 0), stop=(ec == 7))
            nc.vector.tensor_copy(
                T.v[:, sc0:sc0 + 2, :, 0:64],
                pp[:, 0:512].rearrange("p (s h d) -> p s h d", s=2, h=HPC))
        return emit

    if first:
        dma_spans()
        for emit in (u_proj_T(T.wk, T.kT, 0, 0), u_proj_T(T.wk, T.kT, 0, 1),
                     u_proj_T(T.wq, T.qT, 0, 0), u_proj_T(T.wq, T.qT, 0, 1),
                     u_v(0), u_v(2)):
            emit()

    # weave lists; deadlines: kT span z before kc=4z of qc0, v chunk sc
    # before PV kc=sc (PV lags via expp), qT span z before qc z
    weave = {
        0: [u_proj_T(T.wk, T.kT, 1, 0), u_proj_T(T.wk, T.kT, 1, 1),
            u_v(4), u_v(6),
            u_proj_T(T.wk, T.kT, 2, 0), u_proj_T(T.wk, T.kT, 2, 1),
            u_v(8), u_v(10),
            u_proj_T(T.wk, T.kT, 3, 0), u_proj_T(T.wk, T.kT, 3, 1),
            u_v(12), u_v(14),
            u_proj_T(T.wq, T.qT, 1, 0), u_proj_T(T.wq, T.qT, 1, 1)],
        1: [u_proj_T(T.wq, T.qT, 2, 0), u_proj_T(T.wq, T.qT, 2, 1)],
        2: [u_proj_T(T.wq, T.qT, 3, 0), u_proj_T(T.wq, T.qT, 3, 1)],
        3: [] if last else [
            u_proj_T(T.wk, T.kT, 0, 0), u_proj_T(T.wk, T.kT, 0, 1),
            u_proj_T(T.wq, T.qT, 0, 0), u_proj_T(T.wq, T.qT, 0, 1),
            u_v(0), u_v(2)],
    }

    pend = []        # deferred PV work: (qc, kc, exs)
    fz_by_qc = {}

    def norm_and_outproj(fzs, qc):
        outT = T.outp.tile([128, 2, 512], BF16, tag="outT")
        rcs = []
        for hp in range(2):
            rc = T.recp.tile([64, 2, 512], F32, tag="rc")
            nc.vector.reciprocal(rc[:], fzs[hp][64:128, :, :])
            rcs.append(rc)
            for par in range(2):
                nc.vector.tensor_mul(
                    outT[par * 64:(par + 1) * 64, hp, :],
                    fzs[hp][0:64, par, :], rcs[hp][:, par, :])
        for sq in range(4):
            ep = T.fzp.tile([128, 2, 512], F32, tag="fz")
            for half in range(2):
                for hp in range(2):
                    nc.tensor.matmul(
                        ep[:, half, :],
                        outT[:, hp, sq * 128:(sq + 1) * 128],
                        T.wo[:, hp, half * 512:(half + 1) * 512],
                        start=(hp == 0), stop=(hp == 1))
            yt = T.ysb.tile([128, E], F32, tag="y")
            nc.vector.tensor_copy(yt[:], ep.rearrange("p a b -> p (a b)"))
            nc.sync.dma_start(
                T.y_d[(qc * 4 + sq) * 128:(qc * 4 + sq + 1) * 128, :], yt[:])

    def flush_one():
        qc2, kc2, exs2 = pend.pop(0)
        if kc2 == 0:
            fz0 = T.fzp.tile([128, 2, 512], F32, tag="fz")
            fz1 = T.fzp.tile([128, 2, 512], F32, tag="fz")
            fz_by_qc[qc2] = [fz0, fz1]
        fzs2 = fz_by_qc[qc2]
        for hp in range(2):
            for par in range(2):
                h = 2 * hp + par
                nc.tensor.matmul(
                    fzs2[hp][:, par, :],
                    T.v[:, kc2, h, :],
                    exs2[hp][:, par * 512:(par + 1) * 512],
                    start=(kc2 == 0), stop=(kc2 == 15))
        if kc2 == 15:
            norm_and_outproj(fz_by_qc.pop(qc2), qc2)

    for qc in range(4):
        units = weave[qc]
        ui = 0
        for kc in range(16):
            if qc == 3 and kc == 0 and not last:
                dma_spans()   # next rep's x^T; rep-r readers done by qc2
            sts = []
            for hp in range(2):
                st = T.mm.tile([128, 1024], F32, tag="st")
                for par in range(2):   # K=64 row-packed head pair
                    lo, hi = par * 64, (par + 1) * 64
                    nc.tensor.matmul(
                        st[:, par * 512:(par + 1) * 512],
                        T.kT[lo:hi, hp, kc * 128:(kc + 1) * 128],
                        T.qT[lo:hi, hp, qc * 512:(qc + 1) * 512],
                        start=True, stop=True)
                sts.append(st)
            exs = []
            for hp in range(2):
                ex = T.expp.tile([128, 1024], BF16, tag="ex")
                nc.scalar.activation(ex[:], sts[hp][:], EXP, scale=0.125)
                exs.append(ex)
            pend.append((qc, kc, exs))
            if len(pend) > DELTA:
                flush_one()
            if kc >= 1 and ui < len(units):
                units[ui]()
                ui += 1
        while ui < len(units):
            units[ui]()
            ui += 1
    while pend:
        flush_one()


def _get_nc():
    global _cached
    if _cached is None:
        _cached = build()
    return _cached


def _bf16(a):
    import ml_dtypes
    return np.ascontiguousarray(a, dtype=ml_dtypes.bfloat16)


def make_in_maps(x, w_qkv, w_out):
    x = np.asarray(x, dtype=np.float32)
    w_qkv = np.asarray(w_qkv, dtype=np.float32)
    w_out = np.asarray(w_out, dtype=np.float32)
    in_maps = []
    for c in range(N_CORES):
        b, r = c // 4, c % 4
        hs = r * HD                  # first qkv column of this core's heads
        in_maps.append({
            "xT": _bf16(x[b].T),
            "wq": _bf16(w_qkv[:, hs:hs + HD]),
            "wk": _bf16(w_qkv[:, E + hs:E + hs + HD]),
            "wv": _bf16(w_qkv[:, 2 * E + hs:2 * E + hs + HD]),
            "wo": _bf16(w_out[r * HD:(r + 1) * HD, :]),
        })
    return in_maps


def assemble(results):
    y = np.zeros((B, S, E), dtype=np.float32)
    for c in range(N_CORES):
        y[c // 4] += results[c]["y"]
    return y


def kernel(x, w_qkv, w_out):
    nc = _get_nc()
    res = run_bass_kernel_spmd(nc, make_in_maps(x, w_qkv, w_out),
                               list(range(N_CORES)))
    return assemble(res.results)
